# revision 1
# baseline (speedup 1.0000x reference)
"""KSparseFFTClassifier Trainium2 kernel.

Math: reference computes
    h   = x @ W_proj.T + b_proj                      (bs, 129)
    h  *= scale  (sqrt(2) on dims 1..64)
    out = IDFT65(h[:, :65]) + h[:, 65:] @ Ws.T       (bs, 16384)

The zero-padded orthonormal IDFT of the 65 nonzero frequency components is a
dense matmul against a (65, N) cos/sin basis; the DC row of that basis is the
constant 1/sqrt(N).  So with M = [scaled cos/sin basis for h dims 1..64;
Ws.T]  (128 x N):

    out[b, n] = h[b, 1:129] @ M[:, n] + (h[b, 0] + 0) / sqrt(N)

i.e. a (bs,2048)x(2048,128) matmul, a (bs,128)x(128,N) matmul, and a
per-row scalar (the DC term) added during PSUM eviction.

Sharding: data-parallel over batch, 512 rows per core on 8 cores.
"""

import numpy as np

BS = 4096
IN_DIM = 2048
N = 16384
K = 32
SLACK = 64
NCORES = 8
BC = BS // NCORES        # 512 batch rows per core
P = 128
KT = IN_DIM // P         # 16 contraction tiles for matmul1
NCHUNK = 4096            # output column chunk (SBUF out tile free size)
NCH = N // NCHUNK        # 4

# matmul dtypes ("float32" = exact 2-pass fp32, 4 cyc/row;
# "float32r" = single-pass fp32, 1 cyc/row at free>=256)
MM1_DT = "float32r"
MM2_DT = "float32r"

_NC_CACHE = {}


def _build_nc(mm1_name, mm2_name):
    import concourse.bacc as bacc
    import concourse.mybir as mybir
    import concourse.tile as tile

    f32 = mybir.dt.float32
    mm1 = getattr(mybir.dt, mm1_name)
    mm2 = getattr(mybir.dt, mm2_name)

    nc = bacc.Bacc("TRN2", target_bir_lowering=False)

    xT = nc.dram_tensor("xT", [P, KT * BC], mm1, kind="ExternalInput")
    w1t = nc.dram_tensor("w1t", [P, KT * P], mm1, kind="ExternalInput")
    w0 = nc.dram_tensor("w0", [P, KT], f32, kind="ExternalInput")
    mmat = nc.dram_tensor("mmat", [P, N], mm2, kind="ExternalInput")
    bt = nc.dram_tensor("bt", [P, 1], f32, kind="ExternalInput")
    cst = nc.dram_tensor("cst", [1, 1], f32, kind="ExternalInput")
    out = nc.dram_tensor("out", [BC, N], f32, kind="ExternalOutput")

    Ident = mybir.ActivationFunctionType.Identity

    with tile.TileContext(nc) as tc:
        with (
            tc.tile_pool(name="wp", bufs=1) as wp,
            tc.tile_pool(name="xp", bufs=1) as xp,
            tc.tile_pool(name="mp", bufs=1) as mp,
            tc.tile_pool(name="hp", bufs=1) as hp,
            tc.tile_pool(name="op", bufs=3) as op,
            tc.tile_pool(name="ps", bufs=4, space="PSUM") as ps,
            tc.tile_pool(name="ps1", bufs=1, space="PSUM") as ps1,
            tc.tile_pool(name="ps2", bufs=1, space="PSUM") as ps2,
        ):
            w1t_sb = wp.tile([P, KT * P], mm1, tag="w1t")
            nc.sync.dma_start(out=w1t_sb[:, :], in_=w1t[:, :])
            w0_sb = wp.tile([P, KT], f32, tag="w0")
            nc.sync.dma_start(out=w0_sb[:, :], in_=w0[:, :])
            bt_sb = wp.tile([P, 1], f32, tag="bt")
            nc.sync.dma_start(out=bt_sb[:, :], in_=bt[:, :])
            cst_sb = wp.tile([1, 1], f32, tag="cst")
            nc.sync.dma_start(out=cst_sb[:, :], in_=cst[:, :])
            ones_sb = wp.tile([1, 1], f32, tag="ones")
            nc.vector.memset(ones_sb[:, :], 1.0)

            # x transposed, packed on host as 4 groups of 4 k-tiles
            xg = []
            for g in range(4):
                t = xp.tile([P, 4 * BC], mm1, tag=f"xg{g}")
                nc.sync.dma_start(out=t[:, :], in_=xT[:, g * 4 * BC:(g + 1) * 4 * BC])
                xg.append(t)

            # combined IDFT-basis + Ws.T matrix, resident in SBUF
            mm = []
            for ti in range(NCH):
                m = mp.tile([P, NCHUNK], mm2, tag=f"m{ti}")
                nc.sync.dma_start(out=m[:, :], in_=mmat[:, ti * NCHUNK:(ti + 1) * NCHUNK])
                mm.append(m)

            # matmul1: hT[d, b] for d = h dims 1..128
            hT_ps = ps1.tile([P, BC], f32, tag="hT")
            for kt in range(KT):
                nc.tensor.matmul(
                    hT_ps[:, :],
                    lhsT=w1t_sb[:, kt * P:(kt + 1) * P],
                    rhs=xg[kt // 4][:, (kt % 4) * BC:(kt % 4 + 1) * BC],
                    start=(kt == 0),
                    stop=(kt == KT - 1),
                )
            hT_sb = hp.tile([P, BC], mm2, tag="hT_sb")
            nc.scalar.add(hT_sb[:, :], hT_ps[:, :], bt_sb[:, 0:1])

            # dc row: h dim 0 (as (1, BC)), then PE-transpose to (P, 4)
            dcr_ps = ps2.tile([1, BC], f32, tag="dcr")
            for kt in range(KT):
                nc.tensor.matmul(
                    dcr_ps[:, :],
                    lhsT=w0_sb[:, kt:kt + 1],
                    rhs=xg[kt // 4][:, (kt % 4) * BC:(kt % 4 + 1) * BC].bitcast(f32),
                    start=(kt == 0),
                    stop=(kt == KT - 1),
                )
            dcr_sb = hp.tile([1, BC], f32, tag="dcr_sb")
            nc.scalar.activation(
                dcr_sb[:, :], dcr_ps[:, :], Ident,
                bias=cst_sb[0:1, 0:1], scale=float(1.0 / np.sqrt(N)),
            )
            dc_sb = hp.tile([P, BC // P], f32, tag="dc_sb")
            for j in range(BC // P):
                dcc_ps = ps2.tile([P, 1], f32, tag="dcc")
                nc.tensor.matmul(
                    dcc_ps[:, :],
                    lhsT=dcr_sb[0:1, j * P:(j + 1) * P],
                    rhs=ones_sb[0:1, 0:1],
                    start=True,
                    stop=True,
                )
                nc.scalar.copy(dc_sb[:, j:j + 1], dcc_ps[:, :])

            # matmul2 + DC bias-add eviction + store
            ev = 0
            for ti in range(NCH):
                for j in range(BC // P):
                    ob = op.tile([P, NCHUNK], f32, tag="ob")
                    for s in range(NCHUNK // 512):
                        pt = ps.tile([P, 512], f32, tag="mm2")
                        nc.tensor.matmul(
                            pt[:, :],
                            lhsT=hT_sb[:, j * P:(j + 1) * P],
                            rhs=mm[ti][:, s * 512:(s + 1) * 512],
                            start=True,
                            stop=True,
                        )
                        dst = ob[:, s * 512:(s + 1) * 512]
                        if ev % 2 == 0:
                            nc.scalar.add(dst, pt[:, :], dc_sb[:, j:j + 1])
                        else:
                            nc.vector.tensor_scalar_add(dst, pt[:, :], dc_sb[:, j:j + 1])
                        ev += 1
                    nc.sync.dma_start(
                        out=out[j * P:(j + 1) * P, ti * NCHUNK:(ti + 1) * NCHUNK],
                        in_=ob[:, :],
                    )
    nc.compile()
    return nc


def _get_nc():
    key = (MM1_DT, MM2_DT)
    if key not in _NC_CACHE:
        _NC_CACHE[key] = _build_nc(*key)
    return _NC_CACHE[key]


def _host_pack(x, W_proj, b_proj, Ws):
    SQRT2 = np.float64(np.sqrt(np.float32(2.0)))
    n_idx = np.arange(N, dtype=np.float64)
    k_idx = np.arange(1, K + 1, dtype=np.float64)
    theta = (2.0 * np.pi / N) * np.outer(k_idx, n_idx)
    M = np.empty((P, N), np.float32)
    isqn = 1.0 / np.sqrt(np.float64(N))
    M[0:2 * K:2] = (SQRT2 * isqn) * np.cos(theta)
    M[1:2 * K:2] = (SQRT2 * isqn) * np.sin(theta)
    M[2 * K:] = Ws.T

    w1 = W_proj[1:P + 1]                                  # (128, 2048)
    w1t = np.ascontiguousarray(
        w1.T.reshape(KT, P, P).transpose(1, 0, 2).reshape(P, KT * P)
    )
    w0 = np.ascontiguousarray(W_proj[0].reshape(KT, P).T)  # (128, 16)
    bt = np.ascontiguousarray(b_proj[1:P + 1].reshape(P, 1))
    cst = np.asarray(b_proj[0] / np.sqrt(np.float64(N)), np.float32).reshape(1, 1)

    xts = []
    for c in range(NCORES):
        xc = x[c * BC:(c + 1) * BC]                        # (512, 2048)
        xt = np.ascontiguousarray(
            xc.T.reshape(KT, P, BC).transpose(1, 0, 2).reshape(P, KT * BC)
        )
        xts.append(xt)
    return M, w1t, w0, bt, cst, xts


def kernel(x, W_proj, b_proj, Ws, _trace=False, _tmpdir=None):
    from concourse import bass_utils

    x = np.ascontiguousarray(x, np.float32)
    W_proj = np.ascontiguousarray(W_proj, np.float32)
    b_proj = np.ascontiguousarray(b_proj, np.float32)
    Ws = np.ascontiguousarray(Ws, np.float32)

    M, w1t, w0, bt, cst, xts = _host_pack(x, W_proj, b_proj, Ws)
    nc = _get_nc()

    in_maps = [
        {"xT": xts[c], "w1t": w1t, "w0": w0, "mmat": M, "bt": bt, "cst": cst}
        for c in range(NCORES)
    ]
    kw = {}
    if _trace:
        kw = dict(trace=True, tmpdir=_tmpdir, trace_cores=[0])
    res = bass_utils.run_bass_kernel_spmd(nc, in_maps, core_ids=list(range(NCORES)), **kw)
    out = np.concatenate([r["out"] for r in res.results], axis=0)
    if _trace:
        return out, res
    return out



# revision 2
# speedup vs baseline: 1.6756x; 1.6756x over previous
"""KSparseFFTClassifier Trainium2 kernel.

Math: reference computes
    h   = x @ W_proj.T + b_proj                      (bs, 129)
    h  *= scale  (sqrt(2) on dims 1..64)
    out = IDFT65(h[:, :65]) + h[:, 65:] @ Ws.T       (bs, 16384)

The zero-padded orthonormal IDFT of the 65 nonzero frequency components is a
dense matmul against a (65, N) cos/sin basis; the DC row of that basis is the
constant 1/sqrt(N).  So with M = [scaled cos/sin basis for h dims 1..64;
Ws.T]  (128 x N):

    out[b, n] = h[b, 1:129] @ M[:, n] + (h[b, 0] + 0) / sqrt(N)

i.e. a (bs,2048)x(2048,128) matmul, a (bs,128)x(128,N) matmul, and a
per-row scalar (the DC term) added during PSUM eviction.

The kernel is HBM-bound (output store dominates), so all large tensors are
stored in fp16: x/w1t/M as matmul inputs (errors ~1e-3 rel, tolerance 2e-2)
and the output itself (quantization ~1e-4 of output norm).  Host casts the
fp16 result back to fp32.

Sharding: data-parallel over batch, 512 rows per core on 8 cores.
"""

import numpy as np

BS = 4096
IN_DIM = 2048
N = 16384
K = 32
SLACK = 64
NCORES = 8
BC = BS // NCORES        # 512 batch rows per core
P = 128
KT = IN_DIM // P         # 16 contraction tiles for matmul1
NCHUNK = 4096            # output column chunk (SBUF out tile free size)
NCH = N // NCHUNK        # 4

# storage/matmul dtypes for the big tensors
MM1_DT = "float16"       # x, w1t, w0
MM2_DT = "float16"       # hT, M
OUT_DT = "float16"       # output DRAM tensor (host casts back to fp32)

_NC_CACHE = {}


def _np_dt(name):
    if name in ("float32", "float32r"):
        return np.float32
    if name == "float16":
        return np.float16
    if name == "bfloat16":
        import ml_dtypes
        return ml_dtypes.bfloat16
    raise ValueError(name)


def _build_nc(mm1_name, mm2_name, out_name):
    import concourse.bacc as bacc
    import concourse.mybir as mybir
    import concourse.tile as tile

    f32 = mybir.dt.float32
    mm1 = getattr(mybir.dt, mm1_name)
    mm2 = getattr(mybir.dt, mm2_name)
    odt = getattr(mybir.dt, out_name)

    nc = bacc.Bacc("TRN2", target_bir_lowering=False)

    xT = nc.dram_tensor("xT", [P, KT * BC], mm1, kind="ExternalInput")
    w1t = nc.dram_tensor("w1t", [P, KT * P], mm1, kind="ExternalInput")
    w0 = nc.dram_tensor("w0", [P, KT], mm1, kind="ExternalInput")
    mmat = nc.dram_tensor("mmat", [P, N], mm2, kind="ExternalInput")
    bt = nc.dram_tensor("bt", [P, 1], f32, kind="ExternalInput")
    cst = nc.dram_tensor("cst", [1, 1], f32, kind="ExternalInput")
    out = nc.dram_tensor("out", [BC, N], odt, kind="ExternalOutput")

    Ident = mybir.ActivationFunctionType.Identity

    with tile.TileContext(nc) as tc:
        with (
            tc.tile_pool(name="wp", bufs=1) as wp,
            tc.tile_pool(name="xp", bufs=1) as xp,
            tc.tile_pool(name="mp", bufs=1) as mp,
            tc.tile_pool(name="hp", bufs=1) as hp,
            tc.tile_pool(name="op", bufs=3) as op,
            tc.tile_pool(name="ps", bufs=4, space="PSUM") as ps,
            tc.tile_pool(name="ps1", bufs=1, space="PSUM") as ps1,
            tc.tile_pool(name="ps2", bufs=1, space="PSUM") as ps2,
        ):
            w1t_sb = wp.tile([P, KT * P], mm1, tag="w1t")
            nc.sync.dma_start(out=w1t_sb[:, :], in_=w1t[:, :])
            w0_sb = wp.tile([P, KT], mm1, tag="w0")
            nc.sync.dma_start(out=w0_sb[:, :], in_=w0[:, :])
            bt_sb = wp.tile([P, 1], f32, tag="bt")
            nc.sync.dma_start(out=bt_sb[:, :], in_=bt[:, :])
            cst_sb = wp.tile([1, 1], f32, tag="cst")
            nc.sync.dma_start(out=cst_sb[:, :], in_=cst[:, :])
            ones_sb = wp.tile([1, 1], f32, tag="ones")
            nc.vector.memset(ones_sb[:, :], 1.0)

            # x transposed, packed on host as 4 groups of 4 k-tiles
            xg = []
            for g in range(4):
                t = xp.tile([P, 4 * BC], mm1, tag=f"xg{g}")
                nc.sync.dma_start(out=t[:, :], in_=xT[:, g * 4 * BC:(g + 1) * 4 * BC])
                xg.append(t)

            # combined IDFT-basis + Ws.T matrix, resident in SBUF
            mm = []
            for ti in range(NCH):
                m = mp.tile([P, NCHUNK], mm2, tag=f"m{ti}")
                nc.sync.dma_start(out=m[:, :], in_=mmat[:, ti * NCHUNK:(ti + 1) * NCHUNK])
                mm.append(m)

            # matmul1: hT[d, b] for d = h dims 1..128
            hT_ps = ps1.tile([P, BC], f32, tag="hT")
            for kt in range(KT):
                nc.tensor.matmul(
                    hT_ps[:, :],
                    lhsT=w1t_sb[:, kt * P:(kt + 1) * P],
                    rhs=xg[kt // 4][:, (kt % 4) * BC:(kt % 4 + 1) * BC],
                    start=(kt == 0),
                    stop=(kt == KT - 1),
                )
            hT_sb = hp.tile([P, BC], mm2, tag="hT_sb")
            nc.scalar.add(hT_sb[:, :], hT_ps[:, :], bt_sb[:, 0:1])

            # dc row: h dim 0 (as (1, BC)), then PE-transpose to (P, 4)
            dcr_ps = ps2.tile([1, BC], f32, tag="dcr")
            for kt in range(KT):
                nc.tensor.matmul(
                    dcr_ps[:, :],
                    lhsT=w0_sb[:, kt:kt + 1],
                    rhs=xg[kt // 4][:, (kt % 4) * BC:(kt % 4 + 1) * BC],
                    start=(kt == 0),
                    stop=(kt == KT - 1),
                )
            dcr_sb = hp.tile([1, BC], f32, tag="dcr_sb")
            nc.scalar.activation(
                dcr_sb[:, :], dcr_ps[:, :], Ident,
                bias=cst_sb[0:1, 0:1], scale=float(1.0 / np.sqrt(N)),
            )
            dc_sb = hp.tile([P, BC // P], f32, tag="dc_sb")
            for j in range(BC // P):
                dcc_ps = ps2.tile([P, 1], f32, tag="dcc")
                nc.tensor.matmul(
                    dcc_ps[:, :],
                    lhsT=dcr_sb[0:1, j * P:(j + 1) * P],
                    rhs=ones_sb[0:1, 0:1],
                    start=True,
                    stop=True,
                )
                nc.scalar.copy(dc_sb[:, j:j + 1], dcc_ps[:, :])

            # matmul2 + DC bias-add eviction + store
            ev = 0
            for ti in range(NCH):
                for j in range(BC // P):
                    ob = op.tile([P, NCHUNK], odt, tag="ob")
                    for s in range(NCHUNK // 512):
                        pt = ps.tile([P, 512], f32, tag="mm2")
                        nc.tensor.matmul(
                            pt[:, :],
                            lhsT=hT_sb[:, j * P:(j + 1) * P],
                            rhs=mm[ti][:, s * 512:(s + 1) * 512],
                            start=True,
                            stop=True,
                        )
                        dst = ob[:, s * 512:(s + 1) * 512]
                        if ev % 2 == 0:
                            nc.scalar.add(dst, pt[:, :], dc_sb[:, j:j + 1])
                        else:
                            nc.vector.tensor_scalar_add(dst, pt[:, :], dc_sb[:, j:j + 1])
                        ev += 1
                    nc.sync.dma_start(
                        out=out[j * P:(j + 1) * P, ti * NCHUNK:(ti + 1) * NCHUNK],
                        in_=ob[:, :],
                    )
    nc.compile()
    return nc


def _get_nc():
    key = (MM1_DT, MM2_DT, OUT_DT)
    if key not in _NC_CACHE:
        _NC_CACHE[key] = _build_nc(*key)
    return _NC_CACHE[key]


def _host_pack(x, W_proj, b_proj, Ws):
    dt1 = _np_dt(MM1_DT)
    dt2 = _np_dt(MM2_DT)
    SQRT2 = np.float64(np.sqrt(np.float32(2.0)))
    n_idx = np.arange(N, dtype=np.float64)
    k_idx = np.arange(1, K + 1, dtype=np.float64)
    theta = (2.0 * np.pi / N) * np.outer(k_idx, n_idx)
    M = np.empty((P, N), np.float32)
    isqn = 1.0 / np.sqrt(np.float64(N))
    M[0:2 * K:2] = (SQRT2 * isqn) * np.cos(theta)
    M[1:2 * K:2] = (SQRT2 * isqn) * np.sin(theta)
    M[2 * K:] = Ws.T
    M = np.ascontiguousarray(M.astype(dt2))

    w1 = W_proj[1:P + 1]                                  # (128, 2048)
    w1t = np.ascontiguousarray(
        w1.T.reshape(KT, P, P).transpose(1, 0, 2).reshape(P, KT * P).astype(dt1)
    )
    w0 = np.ascontiguousarray(W_proj[0].reshape(KT, P).T.astype(dt1))  # (128, 16)
    bt = np.ascontiguousarray(b_proj[1:P + 1].reshape(P, 1))
    cst = np.asarray(b_proj[0] / np.sqrt(np.float64(N)), np.float32).reshape(1, 1)

    xts = []
    for c in range(NCORES):
        xc = x[c * BC:(c + 1) * BC]                        # (512, 2048)
        xt = np.ascontiguousarray(
            xc.T.reshape(KT, P, BC).transpose(1, 0, 2).reshape(P, KT * BC).astype(dt1)
        )
        xts.append(xt)
    return M, w1t, w0, bt, cst, xts


def kernel(x, W_proj, b_proj, Ws, _trace=False, _tmpdir=None):
    from concourse import bass_utils

    x = np.ascontiguousarray(x, np.float32)
    W_proj = np.ascontiguousarray(W_proj, np.float32)
    b_proj = np.ascontiguousarray(b_proj, np.float32)
    Ws = np.ascontiguousarray(Ws, np.float32)

    M, w1t, w0, bt, cst, xts = _host_pack(x, W_proj, b_proj, Ws)
    nc = _get_nc()

    in_maps = [
        {"xT": xts[c], "w1t": w1t, "w0": w0, "mmat": M, "bt": bt, "cst": cst}
        for c in range(NCORES)
    ]
    kw = {}
    if _trace:
        kw = dict(trace=True, tmpdir=_tmpdir, trace_cores=[0])
    res = bass_utils.run_bass_kernel_spmd(nc, in_maps, core_ids=list(range(NCORES)), **kw)
    out = np.concatenate(
        [np.asarray(r["out"]).astype(np.float32) for r in res.results], axis=0
    )
    if _trace:
        return out, res
    return out


# revision 10
# speedup vs baseline: 1.8488x; 1.1033x over previous
"""KSparseFFTClassifier Trainium2 kernel.

Math: reference computes
    h   = x @ W_proj.T + b_proj                      (bs, 129)
    h  *= scale  (sqrt(2) on dims 1..64)
    out = IDFT65(h[:, :65]) + h[:, 65:] @ Ws.T       (bs, 16384)

The zero-padded orthonormal IDFT of the 65 nonzero frequency components is a
dense matmul against a (65, N) cos/sin basis; the DC row of that basis is the
constant 1/sqrt(N).  So with M = [scaled cos/sin basis for h dims 1..64;
Ws.T]  (128 x N):

    out[b, n] = h[b, 1:129] @ M[:, n] + (h[b, 0] + 0) / sqrt(N)

i.e. a (bs,2048)x(2048,128) matmul, a (bs,128)x(128,N) matmul, and a
per-row scalar (the DC term) added during PSUM eviction.

The kernel is HBM-bound (output store dominates), so storage dtypes are
minimized: fp8e4m3 for x / W1 / M / hT (matmul inputs; each contributes
~0.5% relative error vs the 2e-2 tolerance) and fp16 for the output.
fp8 tensors are rescaled on host into e4m3's normal range (W1 x32, IDFT
basis rows x8, Ws rows x4) and compensated exactly by the per-partition
scale/bias vector applied when h is evicted from PSUM.

Sharding: data-parallel over batch, 512 rows per core on 8 cores.
"""

import numpy as np

BS = 4096
IN_DIM = 2048
N = 16384
K = 32
SLACK = 64
NCORES = 8
BC = BS // NCORES        # 512 batch rows per core
P = 128
KT = IN_DIM // P         # 16 contraction tiles for matmul1
NCHUNK = 4096            # output column chunk (SBUF out tile free size)
NCH = N // NCHUNK        # 4
PSF = 1024               # PSUM tile free size (2 banks); evict granularity

# storage/matmul dtypes for the big tensors
MM1_DT = "float8e4"      # x, w1t, w0
MM2_DT = "float8e4"      # hT, M
OUT_DT = "float16"       # output DRAM tensor (host casts back to fp32)

_NC_CACHE = {}


def _np_dt(name):
    if name in ("float32", "float32r"):
        return np.float32
    if name == "float16":
        return np.float16
    if name == "bfloat16":
        import ml_dtypes
        return ml_dtypes.bfloat16
    if name == "float8e4":
        import ml_dtypes
        return ml_dtypes.float8_e4m3
    raise ValueError(name)


def _scales(mm1_name, mm2_name):
    # host-side rescale factors keeping fp8 values in e4m3 normal range
    w1sc = 32.0 if mm1_name == "float8e4" else 1.0
    if mm2_name == "float8e4":
        sm = np.concatenate([np.full(2 * K, 8.0), np.full(SLACK, 4.0)])
    else:
        sm = np.ones(P)
    return w1sc, sm.astype(np.float64)


def _build_nc(mm1_name, mm2_name, out_name):
    import concourse.bacc as bacc
    import concourse.mybir as mybir
    import concourse.tile as tile

    f32 = mybir.dt.float32
    mm1 = getattr(mybir.dt, mm1_name)
    mm2 = getattr(mybir.dt, mm2_name)
    odt = getattr(mybir.dt, out_name)

    w1sc = 32.0 if mm1_name == "float8e4" else 1.0

    nc = bacc.Bacc("TRN2", target_bir_lowering=False)

    xT = nc.dram_tensor("xT", [P, KT * BC], mm1, kind="ExternalInput")
    w1t = nc.dram_tensor("w1t", [P, KT * P], mm1, kind="ExternalInput")
    w0 = nc.dram_tensor("w0", [P, KT], mm1, kind="ExternalInput")
    mmat = nc.dram_tensor("mmat", [P, N], mm2, kind="ExternalInput")
    # col 0: hT evict scale 1/(w1sc*sm); col 1: hT evict bias bt/sm
    sb2 = nc.dram_tensor("sb2", [P, 2], f32, kind="ExternalInput")
    cst = nc.dram_tensor("cst", [1, 1], f32, kind="ExternalInput")
    out = nc.dram_tensor("out", [BC, N], odt, kind="ExternalOutput")

    Ident = mybir.ActivationFunctionType.Identity

    with tile.TileContext(nc) as tc:
        with (
            tc.tile_pool(name="wp", bufs=1) as wp,
            tc.tile_pool(name="xp", bufs=1) as xp,
            tc.tile_pool(name="mp", bufs=1) as mp,
            tc.tile_pool(name="hp", bufs=1) as hp,
            tc.tile_pool(name="op", bufs=3) as op,
            tc.tile_pool(name="ps", bufs=4, space="PSUM") as ps,
        ):
            w1t_sb = wp.tile([P, KT * P], mm1, tag="w1t")
            nc.sync.dma_start(out=w1t_sb[:, :], in_=w1t[:, :])

            # x transposed, packed on host as 4 groups of 4 k-tiles
            xg = []
            for g in range(4):
                t = xp.tile([P, 4 * BC], mm1, tag=f"xg{g}")
                nc.sync.dma_start(out=t[:, :], in_=xT[:, g * 4 * BC:(g + 1) * 4 * BC])
                xg.append(t)

            w0_sb = wp.tile([P, KT], mm1, tag="w0")
            nc.sync.dma_start(out=w0_sb[:, :], in_=w0[:, :])
            sb2_sb = wp.tile([P, 2], f32, tag="sb2")
            nc.sync.dma_start(out=sb2_sb[:, :], in_=sb2[:, :])
            cst_sb = wp.tile([1, 1], f32, tag="cst")
            nc.sync.dma_start(out=cst_sb[:, :], in_=cst[:, :])
            ones_sb = wp.tile([1, 1], f32, tag="ones")
            nc.vector.memset(ones_sb[:, :], 1.0)

            # combined IDFT-basis + Ws.T matrix, resident in SBUF
            mm = []
            for ti in range(NCH):
                m = mp.tile([P, NCHUNK], mm2, tag=f"m{ti}")
                nc.sync.dma_start(out=m[:, :], in_=mmat[:, ti * NCHUNK:(ti + 1) * NCHUNK])
                mm.append(m)

            # matmul1: hT[d, b] for d = h dims 1..128
            # (hT/dcr/dcc borrow rotating slots of the single PSUM pool)
            hT_t = ps.tile([P, PSF], f32, tag="mm2")
            hT_ps = hT_t[:, 0:BC]
            for kt in range(KT):
                nc.tensor.matmul(
                    hT_ps[:, :],
                    lhsT=w1t_sb[:, kt * P:(kt + 1) * P],
                    rhs=xg[kt // 4][:, (kt % 4) * BC:(kt % 4 + 1) * BC],
                    start=(kt == 0),
                    stop=(kt == KT - 1),
                )
            # hT = psum * (1/(w1sc*sm)) + bt/sm   (undo host rescales, add bias)
            hT_sb = hp.tile([P, BC], mm2, tag="hT_sb")
            nc.scalar.activation(
                hT_sb[:, :], hT_ps[:, :], Ident,
                bias=sb2_sb[:, 1:2], scale=sb2_sb[:, 0:1],
            )

            # dc row: h dim 0 (as (1, BC)), then PE-transpose to (P, 4)
            dcr_t = ps.tile([P, PSF], f32, tag="mm2")
            dcr_ps = dcr_t[0:1, 0:BC]
            for kt in range(KT):
                nc.tensor.matmul(
                    dcr_ps[:, :],
                    lhsT=w0_sb[:, kt:kt + 1],
                    rhs=xg[kt // 4][:, (kt % 4) * BC:(kt % 4 + 1) * BC],
                    start=(kt == 0),
                    stop=(kt == KT - 1),
                )
            dcr_sb = hp.tile([1, BC], f32, tag="dcr_sb")
            nc.scalar.activation(
                dcr_sb[:, :], dcr_ps[:, :], Ident,
                bias=cst_sb[0:1, 0:1], scale=float(1.0 / (w1sc * np.sqrt(N))),
            )
            dc_sb = hp.tile([P, BC // P], f32, tag="dc_sb")
            for j in range(BC // P):
                dcc_t = ps.tile([P, PSF], f32, tag="mm2")
                dcc_ps = dcc_t[:, 0:1]
                nc.tensor.matmul(
                    dcc_ps[:, :],
                    lhsT=dcr_sb[0:1, j * P:(j + 1) * P],
                    rhs=ones_sb[0:1, 0:1],
                    start=True,
                    stop=True,
                )
                nc.scalar.copy(dc_sb[:, j:j + 1], dcc_ps[:, :])

            # matmul2 + DC bias-add eviction + store; j outer so mm2 weights
            # (hT columns) reload only 4x
            ev = 0
            for j in range(BC // P):
                for ti in range(NCH):
                    ob = op.tile([P, NCHUNK], odt, tag="ob")
                    for s in range(NCHUNK // PSF):
                        pt = ps.tile([P, PSF], f32, tag="mm2")
                        for u in range(PSF // 512):
                            nc.tensor.matmul(
                                pt[:, u * 512:(u + 1) * 512],
                                lhsT=hT_sb[:, j * P:(j + 1) * P],
                                rhs=mm[ti][:, s * PSF + u * 512:s * PSF + (u + 1) * 512],
                                start=True,
                                stop=True,
                            )
                        dst = ob[:, s * PSF:(s + 1) * PSF]
                        if ev % 2 == 0:
                            nc.scalar.add(dst, pt[:, :], dc_sb[:, j:j + 1])
                        else:
                            nc.vector.tensor_scalar_add(dst, pt[:, :], dc_sb[:, j:j + 1])
                        ev += 1
                    nc.sync.dma_start(
                        out=out[j * P:(j + 1) * P, ti * NCHUNK:(ti + 1) * NCHUNK],
                        in_=ob[:, :],
                    )
    nc.compile()
    return nc


def _get_nc():
    key = (MM1_DT, MM2_DT, OUT_DT)
    if key not in _NC_CACHE:
        _NC_CACHE[key] = _build_nc(*key)
    return _NC_CACHE[key]


def _host_pack(x, W_proj, b_proj, Ws):
    dt1 = _np_dt(MM1_DT)
    dt2 = _np_dt(MM2_DT)
    w1sc, sm = _scales(MM1_DT, MM2_DT)

    SQRT2 = np.float64(np.sqrt(np.float32(2.0)))
    n_idx = np.arange(N, dtype=np.float64)
    k_idx = np.arange(1, K + 1, dtype=np.float64)
    theta = (2.0 * np.pi / N) * np.outer(k_idx, n_idx)
    M = np.empty((P, N), np.float64)
    isqn = 1.0 / np.sqrt(np.float64(N))
    M[0:2 * K:2] = (SQRT2 * isqn) * np.cos(theta)
    M[1:2 * K:2] = (SQRT2 * isqn) * np.sin(theta)
    M[2 * K:] = Ws.T
    M *= sm[:, None]
    M = np.ascontiguousarray(M.astype(np.float32).astype(dt2))

    w1 = W_proj[1:P + 1].astype(np.float64) * w1sc        # (128, 2048)
    w1t = np.ascontiguousarray(
        w1.T.reshape(KT, P, P).transpose(1, 0, 2).reshape(P, KT * P)
        .astype(np.float32).astype(dt1)
    )
    w0 = np.ascontiguousarray(
        (W_proj[0].astype(np.float64) * w1sc).reshape(KT, P).T
        .astype(np.float32).astype(dt1)
    )
    # hT evict: out = psum * (1/(w1sc*sm)) + bt/sm
    sb2 = np.stack(
        [1.0 / (w1sc * sm), b_proj[1:P + 1].astype(np.float64) / sm], axis=1
    ).astype(np.float32)
    sb2 = np.ascontiguousarray(sb2)
    cst = np.asarray(b_proj[0] / np.sqrt(np.float64(N)), np.float32).reshape(1, 1)

    xts = []
    for c in range(NCORES):
        xc = x[c * BC:(c + 1) * BC]                        # (512, 2048)
        xt = np.ascontiguousarray(
            xc.T.reshape(KT, P, BC).transpose(1, 0, 2).reshape(P, KT * BC).astype(dt1)
        )
        xts.append(xt)
    return M, w1t, w0, sb2, cst, xts


def kernel(x, W_proj, b_proj, Ws, _trace=False, _tmpdir=None):
    from concourse import bass_utils

    x = np.ascontiguousarray(x, np.float32)
    W_proj = np.ascontiguousarray(W_proj, np.float32)
    b_proj = np.ascontiguousarray(b_proj, np.float32)
    Ws = np.ascontiguousarray(Ws, np.float32)

    M, w1t, w0, sb2, cst, xts = _host_pack(x, W_proj, b_proj, Ws)
    nc = _get_nc()

    in_maps = [
        {"xT": xts[c], "w1t": w1t, "w0": w0, "mmat": M, "sb2": sb2, "cst": cst}
        for c in range(NCORES)
    ]
    kw = {}
    if _trace:
        kw = dict(trace=True, tmpdir=_tmpdir, trace_cores=[0])
    res = bass_utils.run_bass_kernel_spmd(nc, in_maps, core_ids=list(range(NCORES)), **kw)
    out = np.concatenate(
        [np.asarray(r["out"]).astype(np.float32) for r in res.results], axis=0
    )
    if _trace:
        return out, res
    return out
